# revision 29
# baseline (speedup 1.0000x reference)
"""Atomwise (segment_reduce) Trainium2 kernel.

y[m] = sum_{atoms i in molecule m} (x[i] . W[0] + b[0]),  m in [0, 100000)

8 NeuronCores, SPMD (one shared graph), no collectives.  The bias is
folded into x on the host (x += W0*b0/|W0|^2), so the device computes a
pure projected segment-sum.  x ships as fp8 e3m4 (half the HBM bytes of
bf16; rel-err ~1.1e-2 on this data, inside the 2e-2 gate).

Fixed geometry on every core: 250_000 atoms split into NCH=123 windows
of 2048 atoms (16 blocks of 128).  Windows/blocks cut mid-molecule; a
molecule split across blocks/windows/cores is re-summed on the host
during unpack (np.add.at over ~115k (slot -> molecule) entries).

Device pipeline, pair-batched (2 chunks = 4096 atoms per batch) to
halve DVE/ScalarE op overhead and semaphore traffic:
  * grouped DMA of fp8 X windows, 8 chunks per dma_start (16.4 KB
    contiguous per partition), 10 buffers of lookahead
  * one DVE is_equal per PAIR builds H[atom, (block,slot)]:
    block-relative molecule one-hot, BF=12 slots/block, lidx broadcast
    vs iota (emitted ~5 pairs ahead, 6-deep pair pool)
  * 16 matmuls per chunk: stationary X_j [128 atoms x 128 feats] fp8
    (FWL), moving H_j [128 x 12] bf16 -> PSUM S^T[128 feats, 384
    slots/pair], disjoint per-block slices, ~27 ns/pair issue rate
  * one ScalarE copy per pair: S^T -> SBUF bf16
  * 3 projection matmuls per pair (delayed 1 pair): y = S_sb^T @ w0
    written straight into one persistent PSUM bank holding the entire
    [128, 185] output (slots linear: g = block*BF + s -> (g//128 col,
    g%128 partition)); single copy + DMA out at the end
"""

import numpy as np
import ml_dtypes

N_ATOMS = 2_000_000
N_IN = 128
N_MOL = 100_000
NCORES = 8
P = 128
PC = N_ATOMS // NCORES      # atoms per core
NB = 16                     # 128-atom blocks per window
A_max = NB * P              # 2048 atoms per window
NCH = (PC + A_max - 1) // A_max   # 123 windows
NPAD = NCH * A_max
BF = 12                     # molecule slots per block (max span 10 on data)
SLOTS = NB * BF             # 192 slots per chunk
PAIRW = 2 * SLOTS           # 384 slots per chunk-pair (= 3 * 128)
NPAIR = (NCH + 1) // 2
NCOL = (NCH * SLOTS + P - 1) // P   # 185 output columns

_graph_cache: dict = {}


def _chunk_groups(nch):
    groups, c = [], 0
    for sz in (8, 4, 2, 1):
        while nch - c >= sz:
            groups.append((c, sz))
            c += sz
    return groups


def _build_graph():
    import concourse.mybir as mybir
    from concourse import bacc
    from concourse.tile import TileContext

    f32 = mybir.dt.float32
    bf16 = mybir.dt.bfloat16
    f8 = mybir.dt.float8e3

    IOTA_OFF = 0
    LIDX_OFF = PAIRW
    W0_OFF = LIDX_OFF + NCH * NB
    W0_OFF += W0_OFF % 2
    CW = W0_OFF + 4

    nc = bacc.Bacc()
    xw = nc.dram_tensor("xw", [NPAD, N_IN], f8, kind="ExternalInput")
    cst = nc.dram_tensor("cst", [P, CW], f8, kind="ExternalInput")
    out = nc.dram_tensor("out", [P * NCOL], f32, kind="ExternalOutput")
    out_r = out.rearrange("(p c) -> p c", c=NCOL)
    groups = _chunk_groups(NCH)

    with TileContext(nc) as tc:
        with tc.tile_pool(name="const", bufs=1) as cpool, \
             tc.tile_pool(name="xbp", bufs=10) as xbpool, \
             tc.tile_pool(name="hp", bufs=6) as hpool, \
             tc.tile_pool(name="scp", bufs=4) as scpool, \
             tc.tile_pool(name="pp", bufs=5, space="PSUM") as pspool, \
             tc.tile_pool(name="yp", bufs=1, space="PSUM") as ypool:
            cst_t = cpool.tile([P, CW], f8)
            yp_all = ypool.tile([P, 512], f32)
            w0_col = cst_t[:, W0_OFF:W0_OFF + 2].bitcast(bf16)

            EQ_LEAD_PAIRS = 4
            eq_pairs = {}

            def _emit_eq(pr):
                w = min(PAIRW, (NCH - 2 * pr) * SLOTS)
                ht = hpool.tile([P, PAIRW], bf16, tag="h")
                eq_pairs[pr] = ht
                for o in range(0, w, SLOTS):
                    nc.vector.tensor_tensor(
                        out=ht[:, o:o + SLOTS],
                        in0=cst_t[:, LIDX_OFF + pr * 2 * NB + o // BF:
                                  LIDX_OFF + pr * 2 * NB + (o + SLOTS) // BF
                                  ].to_broadcast([P, NB, BF]),
                        in1=cst_t[:, IOTA_OFF:IOTA_OFF + SLOTS],
                        op=mybir.AluOpType.is_equal)

            def _emit_proj(pr, sc, w):
                base = pr * PAIRW
                o = 0
                while o < w:
                    m = min(P, w - o)
                    nc.tensor.matmul(
                        yp_all[0:m, (base + o) // P:(base + o) // P + 1],
                        lhsT=sc[:, o:o + m],
                        rhs=w0_col[:, 0:1],
                        start=True,
                        stop=True,
                    )
                    o += m

            chunk_xq = {}
            for gstart, gc in groups:
                for cc in range(gc):
                    chunk_xq[gstart + cc] = (gstart, gc, cc)

            group_tiles = {}
            proj_q = []
            ps_cur = None
            for c in range(NCH):
                gstart, gc, cc = chunk_xq[c]
                if cc == 0:
                    if gstart == 0:
                        nc.sync.dma_start(cst_t[:], cst[:, :])
                    xq = xbpool.tile([P, gc * NB * N_IN], f8, tag="xq")
                    group_tiles[gstart] = xq
                    nc.sync.dma_start(
                        xq[:],
                        xw[gstart * A_max:(gstart + gc) * A_max, :].rearrange(
                            "(p j) f -> p (j f)", p=P),
                    )
                    if gstart == 0:
                        for pe in range(min(EQ_LEAD_PAIRS + 1, NPAIR)):
                            _emit_eq(pe)
                xq = group_tiles[gstart]
                pr = c // 2
                off = (c % 2) * SLOTS
                if off == 0:
                    ps_cur = pspool.tile([P, 512], f32, tag="ps")
                ht = eq_pairs[pr]
                for j in range(NB):
                    nc.tensor.matmul(
                        ps_cur[:, off + j * BF:off + (j + 1) * BF],
                        lhsT=xq[:, (cc * NB + j) * N_IN:
                                (cc * NB + j + 1) * N_IN],
                        rhs=ht[:, off + j * BF:off + (j + 1) * BF],
                        start=True,
                        stop=True,
                    )
                if c % 2 == 1 or c == NCH - 1:
                    w = off + SLOTS
                    sc = scpool.tile([P, PAIRW], bf16, tag="sc")
                    nc.scalar.activation(
                        sc[:, 0:w], ps_cur[:, 0:w],
                        mybir.ActivationFunctionType.Copy)
                    del eq_pairs[pr]
                    if pr + EQ_LEAD_PAIRS + 1 < NPAIR:
                        _emit_eq(pr + EQ_LEAD_PAIRS + 1)
                    proj_q.append((pr, sc, w))
                    if len(proj_q) > 1:
                        _emit_proj(*proj_q.pop(0))
            while proj_q:
                _emit_proj(*proj_q.pop(0))
            y_sb = cpool.tile([P, NCOL], f32)
            nc.vector.tensor_copy(y_sb[:], yp_all[:, 0:NCOL])
            nc.sync.dma_start(out_r[:, :], y_sb[:])
    nc.finalize()
    return nc


def _prep(inputs):
    x = np.asarray(inputs["scalar_representation"], dtype=np.float32)
    idx = np.asarray(inputs["idx_m"]).astype(np.int64)
    W = np.asarray(inputs["W"], dtype=np.float32)
    b = np.asarray(inputs["b"], dtype=np.float32)

    # fold the bias into x: (x + v) . w0 == x . w0 + b0
    v = W[0] * (b[0] / np.dot(W[0], W[0]))

    IOTA_OFF = 0
    LIDX_OFF = PAIRW
    W0_OFF = LIDX_OFF + NCH * NB
    W0_OFF += W0_OFF % 2
    CW = W0_OFF + 4
    iota_row = np.tile(np.arange(BF, dtype=np.float32), 2 * NB).astype(
        ml_dtypes.float8_e3m4)
    groups = _chunk_groups(NCH)

    in_maps = []
    unpack = []
    for i in range(NCORES):
        idxc = idx[i * PC:(i + 1) * PC]
        q8 = np.zeros((NPAD, N_IN), dtype=ml_dtypes.float8_e3m4)
        q8[:PC] = (x[i * PC:(i + 1) * PC] + v).astype(ml_dtypes.float8_e3m4)

        kb = np.arange(NCH * NB, dtype=np.int64) * P
        valid = kb < PC
        base = np.zeros(NCH * NB, dtype=np.int64)
        base[valid] = idxc[kb[valid]]
        ke = np.minimum(kb + P - 1, PC - 1)
        span = np.zeros(NCH * NB, dtype=np.int64)
        span[valid] = idxc[ke[valid]] - base[valid] + 1
        assert span.max() <= BF, f"block span {span.max()} > BF={BF}"

        lidx = np.full(NPAD, -1.0, dtype=np.float32)
        lidx[:PC] = (idxc - np.repeat(base, P)[:PC]).astype(np.float32)
        lidx_t = lidx.reshape(NCH, NB, P).transpose(2, 0, 1).reshape(
            P, NCH * NB).astype(ml_dtypes.float8_e3m4)

        parts = []
        for gstart, gc in groups:
            blk = q8[gstart * A_max:(gstart + gc) * A_max]
            parts.append(np.ascontiguousarray(
                blk.reshape(gc, NB, P, N_IN).transpose(2, 0, 1, 3)
                   .reshape(gc * A_max, N_IN)))
        xw_i = np.concatenate(parts, axis=0)

        cst = np.zeros((P, CW), dtype=ml_dtypes.float8_e3m4)
        cst[:, IOTA_OFF:IOTA_OFF + PAIRW] = iota_row[None, :]
        cst[:, LIDX_OFF:LIDX_OFF + NCH * NB] = lidx_t
        w0b = W[0].astype(ml_dtypes.bfloat16)[:, None].view(
            ml_dtypes.float8_e3m4)
        cst[:, W0_OFF:W0_OFF + 2] = w0b
        in_maps.append({"xw": xw_i, "cst": np.ascontiguousarray(cst)})

        # unpack tables: slot index g = block*BF + s -> (g//P col, g%P row)
        nblk = int(valid.sum())
        sp = span[:nblk]
        tot = int(sp.sum())
        starts = np.zeros(nblk, dtype=np.int64)
        starts[1:] = np.cumsum(sp)[:-1]
        blk_of = np.repeat(np.arange(nblk, dtype=np.int64), sp)
        s_off = np.arange(tot, dtype=np.int64) - np.repeat(starts, sp)
        mol = np.repeat(base[:nblk], sp) + s_off
        g = blk_of * BF + s_off
        p_idx = g % P
        col_idx = g // P
        unpack.append((p_idx, col_idx, mol))
    return in_maps, unpack


def _run(inputs, trace=False):
    from concourse import bass_utils

    in_maps, unpack = _prep(inputs)
    key = (NCH, BF)
    if key not in _graph_cache:
        _graph_cache[key] = _build_graph()
    nc = _graph_cache[key]

    res = bass_utils.run_bass_kernel_spmd(
        nc, in_maps, core_ids=list(range(NCORES)), trace=trace
    )
    y = np.zeros(N_MOL, dtype=np.float32)
    for i in range(NCORES):
        arr = res.results[i]["out"].reshape(P, NCOL)
        p_idx, col_idx, mol = unpack[i]
        np.add.at(y, mol, arr[p_idx, col_idx])
    return y, res


def kernel(**inputs) -> np.ndarray:
    y, _ = _run(inputs, trace=False)
    return y


# revision 30
# speedup vs baseline: 1.0645x; 1.0645x over previous
"""Atomwise (segment_reduce) Trainium2 kernel.

y[m] = sum_{atoms i in molecule m} (x[i] . W[0] + b[0]),  m in [0, 100000)

8 NeuronCores, SPMD (one shared graph), no collectives.  The bias is
folded into x on the host (x += W0*b0/|W0|^2), so the device computes a
pure projected segment-sum.  x ships as fp8 e3m4 (half the HBM bytes of
bf16; rel-err ~1.1e-2 on this data, inside the 2e-2 gate).

Fixed geometry on every core: 250_000 atoms split into NCH=123 windows
of 2048 atoms (16 blocks of 128).  Windows/blocks cut mid-molecule; a
molecule split across blocks/windows/cores is re-summed on the host
during unpack (np.add.at over ~115k (slot -> molecule) entries).

Device pipeline per 2048-atom chunk:
  * grouped DMA of fp8 X windows, 8 chunks per dma_start (16.4 KB
    contiguous per partition), 10 buffers of lookahead
  * DVE is_equal builds H[atom, (block,slot)]: block-relative molecule
    one-hot, BF=16 slots/block, lidx broadcast vs iota (emitted 8
    chunks ahead of consumption, 12-deep tile pool)
  * 16 matmuls: stationary X_j [128 atoms x 128 feats] fp8 (FWL),
    moving H_j [128 x 16] bf16 -> PSUM S^T[128 feats, 256 slots],
    disjoint per-block slices, ~27 ns/pair issue rate
  * ScalarE copies S^T -> SBUF bf16
  * 2 projection matmuls (delayed 2 chunks): y_slots = S_sb^T @ w0
    written straight into one persistent PSUM bank holding the entire
    [128, 246] output; single copy + DMA out at the end
"""

import numpy as np
import ml_dtypes

N_ATOMS = 2_000_000
N_IN = 128
N_MOL = 100_000
NCORES = 8
P = 128
PC = N_ATOMS // NCORES      # atoms per core
NB = 16                     # 128-atom blocks per window
A_max = NB * P              # 2048 atoms per window
NCH = (PC + A_max - 1) // A_max   # 123 windows
NPAD = NCH * A_max
BF = 16                     # molecule slots per block (max span 10 on data)
SLOTS = NB * BF             # 256 slots per chunk
NYC = SLOTS // P            # 2 projection matmuls / output cols per chunk

_graph_cache: dict = {}


def _chunk_groups(nch):
    groups, c = [], 0
    for sz in (8, 4, 2, 1):
        while nch - c >= sz:
            groups.append((c, sz))
            c += sz
    return groups


def _build_graph():
    import concourse.mybir as mybir
    from concourse import bacc
    from concourse.tile import TileContext

    f32 = mybir.dt.float32
    bf16 = mybir.dt.bfloat16
    f8 = mybir.dt.float8e3

    IOTA_OFF = 0
    LIDX_OFF = SLOTS
    W0_OFF = LIDX_OFF + NCH * NB
    W0_OFF += W0_OFF % 2
    CW = W0_OFF + 4

    nc = bacc.Bacc()
    xw = nc.dram_tensor("xw", [NPAD, N_IN], f8, kind="ExternalInput")
    cst = nc.dram_tensor("cst", [P, CW], f8, kind="ExternalInput")
    out = nc.dram_tensor("out", [P * NCH * NYC], f32, kind="ExternalOutput")
    out_r = out.rearrange("(p c) -> p c", c=NCH * NYC)
    groups = _chunk_groups(NCH)

    with TileContext(nc) as tc:
        with tc.tile_pool(name="const", bufs=1) as cpool, \
             tc.tile_pool(name="xbp", bufs=10) as xbpool, \
             tc.tile_pool(name="hp", bufs=12) as hpool, \
             tc.tile_pool(name="scp", bufs=6) as scpool, \
             tc.tile_pool(name="pp", bufs=5, space="PSUM") as pspool, \
             tc.tile_pool(name="yp", bufs=1, space="PSUM") as ypool:
            cst_t = cpool.tile([P, CW], f8)
            yp_all = ypool.tile([P, 512], f32)
            w0_col = cst_t[:, W0_OFF:W0_OFF + 2].bitcast(bf16)

            PROJ_DELAY = 2
            EQ_LEAD = 8
            ht_tiles = {}

            def _emit_eq(c):
                ht = hpool.tile([P, SLOTS], bf16, tag="h")
                ht_tiles[c] = ht
                nc.vector.tensor_tensor(
                    out=ht[:],
                    in0=cst_t[:, LIDX_OFF + c * NB:
                              LIDX_OFF + (c + 1) * NB
                              ].to_broadcast([P, NB, BF]),
                    in1=cst_t[:, IOTA_OFF:IOTA_OFF + SLOTS],
                    op=mybir.AluOpType.is_equal)

            def _emit_proj(c, sc):
                for k in range(NYC):
                    nc.tensor.matmul(
                        yp_all[:, c * NYC + k:c * NYC + k + 1],
                        lhsT=sc[:, k * P:(k + 1) * P],
                        rhs=w0_col[:, 0:1],
                        start=True,
                        stop=True,
                    )

            chunk_xq = {}
            for gstart, gc in groups:
                xq = None  # placeholder; created at emission time
                for cc in range(gc):
                    chunk_xq[gstart + cc] = (gstart, gc, cc)

            group_tiles = {}
            proj_q = []
            gidx = 0
            for c in range(NCH):
                gstart, gc, cc = chunk_xq[c]
                if cc == 0:
                    if gstart == 0:
                        nc.sync.dma_start(cst_t[:], cst[:, :])
                    xq = xbpool.tile([P, gc * NB * N_IN], f8, tag="xq")
                    group_tiles[gstart] = xq
                    nc.sync.dma_start(
                        xq[:],
                        xw[gstart * A_max:(gstart + gc) * A_max, :].rearrange(
                            "(p j) f -> p (j f)", p=P),
                    )
                    gidx += 1
                    if gstart == 0:
                        for ce in range(min(EQ_LEAD + 1, NCH)):
                            _emit_eq(ce)
                xq = group_tiles[gstart]
                ht = ht_tiles.pop(c)
                ps = pspool.tile([P, 512], f32, tag="ps")
                for j in range(NB):
                    nc.tensor.matmul(
                        ps[:, j * BF:(j + 1) * BF],
                        lhsT=xq[:, (cc * NB + j) * N_IN:
                                (cc * NB + j + 1) * N_IN],
                        rhs=ht[:, j * BF:(j + 1) * BF],
                        start=True,
                        stop=True,
                    )
                sc = scpool.tile([P, SLOTS], bf16, tag="sc")
                nc.scalar.activation(
                    sc[:], ps[:, 0:SLOTS],
                    mybir.ActivationFunctionType.Copy)
                if c + EQ_LEAD + 1 < NCH:
                    _emit_eq(c + EQ_LEAD + 1)
                proj_q.append((c, sc))
                if len(proj_q) > PROJ_DELAY:
                    _emit_proj(*proj_q.pop(0))
            while proj_q:
                _emit_proj(*proj_q.pop(0))
            y_sb = cpool.tile([P, NCH * NYC], f32)
            nc.vector.tensor_copy(y_sb[:], yp_all[:, 0:NCH * NYC])
            nc.sync.dma_start(out_r[:, :], y_sb[:])
    nc.finalize()
    return nc


def _prep(inputs):
    x = np.asarray(inputs["scalar_representation"], dtype=np.float32)
    idx = np.asarray(inputs["idx_m"]).astype(np.int64)
    W = np.asarray(inputs["W"], dtype=np.float32)
    b = np.asarray(inputs["b"], dtype=np.float32)

    # fold the bias into x: (x + v) . w0 == x . w0 + b0
    v = W[0] * (b[0] / np.dot(W[0], W[0]))

    IOTA_OFF = 0
    LIDX_OFF = SLOTS
    W0_OFF = LIDX_OFF + NCH * NB
    W0_OFF += W0_OFF % 2
    CW = W0_OFF + 4
    iota_row = np.tile(np.arange(BF, dtype=np.float32), NB).astype(
        ml_dtypes.float8_e3m4)
    groups = _chunk_groups(NCH)

    in_maps = []
    unpack = []
    for i in range(NCORES):
        idxc = idx[i * PC:(i + 1) * PC]
        q8 = np.zeros((NPAD, N_IN), dtype=ml_dtypes.float8_e3m4)
        q8[:PC] = (x[i * PC:(i + 1) * PC] + v).astype(ml_dtypes.float8_e3m4)

        kb = np.arange(NCH * NB, dtype=np.int64) * P
        valid = kb < PC
        base = np.zeros(NCH * NB, dtype=np.int64)
        base[valid] = idxc[kb[valid]]
        ke = np.minimum(kb + P - 1, PC - 1)
        span = np.zeros(NCH * NB, dtype=np.int64)
        span[valid] = idxc[ke[valid]] - base[valid] + 1
        assert span.max() <= BF, f"block span {span.max()} > BF={BF}"

        lidx = np.full(NPAD, -1.0, dtype=np.float32)
        lidx[:PC] = (idxc - np.repeat(base, P)[:PC]).astype(np.float32)
        lidx_t = lidx.reshape(NCH, NB, P).transpose(2, 0, 1).reshape(
            P, NCH * NB).astype(ml_dtypes.float8_e3m4)

        parts = []
        for gstart, gc in groups:
            blk = q8[gstart * A_max:(gstart + gc) * A_max]
            parts.append(np.ascontiguousarray(
                blk.reshape(gc, NB, P, N_IN).transpose(2, 0, 1, 3)
                   .reshape(gc * A_max, N_IN)))
        xw_i = np.concatenate(parts, axis=0)

        cst = np.zeros((P, CW), dtype=ml_dtypes.float8_e3m4)
        cst[:, IOTA_OFF:IOTA_OFF + SLOTS] = iota_row[None, :]
        cst[:, LIDX_OFF:LIDX_OFF + NCH * NB] = lidx_t
        w0b = W[0].astype(ml_dtypes.bfloat16)[:, None].view(
            ml_dtypes.float8_e3m4)
        cst[:, W0_OFF:W0_OFF + 2] = w0b
        in_maps.append({"xw": xw_i, "cst": np.ascontiguousarray(cst)})

        # unpack tables: (partition, column, molecule) per live slot
        nblk = int(valid.sum())
        sp = span[:nblk]
        tot = int(sp.sum())
        starts = np.zeros(nblk, dtype=np.int64)
        starts[1:] = np.cumsum(sp)[:-1]
        blk_of = np.repeat(np.arange(nblk, dtype=np.int64), sp)
        s_off = np.arange(tot, dtype=np.int64) - np.repeat(starts, sp)
        mol = np.repeat(base[:nblk], sp) + s_off
        slotfull = (blk_of % NB) * BF + s_off
        chunkc = blk_of // NB
        p_idx = slotfull % P
        col_idx = chunkc * NYC + slotfull // P
        unpack.append((p_idx, col_idx, mol))
    return in_maps, unpack


def _run(inputs, trace=False):
    from concourse import bass_utils

    in_maps, unpack = _prep(inputs)
    key = (NCH, BF)
    if key not in _graph_cache:
        _graph_cache[key] = _build_graph()
    nc = _graph_cache[key]

    res = bass_utils.run_bass_kernel_spmd(
        nc, in_maps, core_ids=list(range(NCORES)), trace=trace
    )
    y = np.zeros(N_MOL, dtype=np.float32)
    for i in range(NCORES):
        arr = res.results[i]["out"].reshape(P, NCH * NYC)
        p_idx, col_idx, mol = unpack[i]
        np.add.at(y, mol, arr[p_idx, col_idx])
    return y, res


def kernel(**inputs) -> np.ndarray:
    y, _ = _run(inputs, trace=False)
    return y


# revision 31
# speedup vs baseline: 1.0681x; 1.0034x over previous
"""Atomwise (segment_reduce) Trainium2 kernel.

y[m] = sum_{atoms i in molecule m} (x[i] . W[0] + b[0]),  m in [0, 100000)

8 NeuronCores, SPMD (one shared graph), no collectives.  The bias is
folded into x on the host (x += W0*b0/|W0|^2), so the device computes a
pure projected segment-sum.  x ships as fp8 e3m4 (half the HBM bytes of
bf16; rel-err ~1.1e-2 on this data, inside the 2e-2 gate).

Fixed geometry on every core: 250_000 atoms split into NCH=123 windows
of 2048 atoms (16 blocks of 128).  Windows/blocks cut mid-molecule; a
molecule split across blocks/windows/cores is re-summed on the host
during unpack (np.add.at over ~115k (slot -> molecule) entries).

Device pipeline per 2048-atom chunk:
  * grouped DMA of fp8 X windows, 8 chunks per dma_start (16.4 KB
    contiguous per partition), 10 buffers of lookahead
  * DVE is_equal builds H[atom, (block,slot)]: block-relative molecule
    one-hot, BF=16 slots/block, lidx broadcast vs iota (emitted 8
    chunks ahead of consumption, 12-deep tile pool)
  * 16 matmuls: stationary X_j [128 atoms x 128 feats] fp8 (FWL),
    moving H_j [128 x 16] bf16 -> PSUM S^T[128 feats, 256 slots],
    disjoint per-block slices, ~27 ns/pair issue rate
  * ScalarE copies S^T -> SBUF bf16
  * 2 projection matmuls (delayed 2 chunks): y_slots = S_sb^T @ w0
    written straight into one persistent PSUM bank holding the entire
    [128, 246] output; single copy + DMA out at the end
"""

import numpy as np
import ml_dtypes

N_ATOMS = 2_000_000
N_IN = 128
N_MOL = 100_000
NCORES = 8
P = 128
PC = N_ATOMS // NCORES      # atoms per core
NB = 16                     # 128-atom blocks per window
A_max = NB * P              # 2048 atoms per window
NCH = (PC + A_max - 1) // A_max   # 123 windows
NPAD = NCH * A_max
BF = 16                     # molecule slots per block (max span 10 on data)
SLOTS = NB * BF             # 256 slots per chunk
NYC = SLOTS // P            # 2 projection matmuls / output cols per chunk

_graph_cache: dict = {}


def _chunk_groups(nch):
    groups, c = [], 0
    if nch >= 4:
        groups.append((0, 4))
        c = 4
    for sz in (8, 4, 2, 1):
        while nch - c >= sz:
            groups.append((c, sz))
            c += sz
    return groups


def _build_graph():
    import concourse.mybir as mybir
    from concourse import bacc
    from concourse.tile import TileContext

    f32 = mybir.dt.float32
    bf16 = mybir.dt.bfloat16
    f8 = mybir.dt.float8e3

    IOTA_OFF = 0
    LIDX_OFF = SLOTS
    W0_OFF = LIDX_OFF + NCH * NB
    W0_OFF += W0_OFF % 2
    CW = W0_OFF + 4

    nc = bacc.Bacc()
    xw = nc.dram_tensor("xw", [NPAD, N_IN], f8, kind="ExternalInput")
    cst = nc.dram_tensor("cst", [P, CW], f8, kind="ExternalInput")
    out = nc.dram_tensor("out", [P * NCH * NYC], f32, kind="ExternalOutput")
    out_r = out.rearrange("(p c) -> p c", c=NCH * NYC)
    groups = _chunk_groups(NCH)

    with TileContext(nc) as tc:
        with tc.tile_pool(name="const", bufs=1) as cpool, \
             tc.tile_pool(name="xbp", bufs=10) as xbpool, \
             tc.tile_pool(name="hp", bufs=12) as hpool, \
             tc.tile_pool(name="scp", bufs=6) as scpool, \
             tc.tile_pool(name="pp", bufs=7, space="PSUM") as pspool, \
             tc.tile_pool(name="yp", bufs=1, space="PSUM") as ypool:
            cst_t = cpool.tile([P, CW], f8)
            yp_all = ypool.tile([P, 512], f32)
            w0_col = cst_t[:, W0_OFF:W0_OFF + 2].bitcast(bf16)

            PROJ_DELAY = 2
            EQ_LEAD = 8
            ht_tiles = {}

            def _emit_eq(c):
                ht = hpool.tile([P, SLOTS], bf16, tag="h")
                ht_tiles[c] = ht
                nc.vector.tensor_tensor(
                    out=ht[:],
                    in0=cst_t[:, LIDX_OFF + c * NB:
                              LIDX_OFF + (c + 1) * NB
                              ].to_broadcast([P, NB, BF]),
                    in1=cst_t[:, IOTA_OFF:IOTA_OFF + SLOTS],
                    op=mybir.AluOpType.is_equal)

            def _emit_proj(c, sc):
                for k in range(NYC):
                    nc.tensor.matmul(
                        yp_all[:, c * NYC + k:c * NYC + k + 1],
                        lhsT=sc[:, k * P:(k + 1) * P],
                        rhs=w0_col[:, 0:1],
                        start=True,
                        stop=True,
                    )

            chunk_xq = {}
            for gstart, gc in groups:
                xq = None  # placeholder; created at emission time
                for cc in range(gc):
                    chunk_xq[gstart + cc] = (gstart, gc, cc)

            group_tiles = {}
            proj_q = []
            gidx = 0
            for c in range(NCH):
                gstart, gc, cc = chunk_xq[c]
                if cc == 0:
                    if gstart == 0:
                        nc.sync.dma_start(cst_t[:], cst[:, :])
                    xq = xbpool.tile([P, gc * NB * N_IN], f8, tag="xq")
                    group_tiles[gstart] = xq
                    nc.sync.dma_start(
                        xq[:],
                        xw[gstart * A_max:(gstart + gc) * A_max, :].rearrange(
                            "(p j) f -> p (j f)", p=P),
                    )
                    gidx += 1
                    if gstart == 0:
                        for ce in range(min(EQ_LEAD + 1, NCH)):
                            _emit_eq(ce)
                xq = group_tiles[gstart]
                ht = ht_tiles.pop(c)
                ps = pspool.tile([P, 512], f32, tag="ps")
                for j in range(NB):
                    nc.tensor.matmul(
                        ps[:, j * BF:(j + 1) * BF],
                        lhsT=xq[:, (cc * NB + j) * N_IN:
                                (cc * NB + j + 1) * N_IN],
                        rhs=ht[:, j * BF:(j + 1) * BF],
                        start=True,
                        stop=True,
                    )
                sc = scpool.tile([P, SLOTS], bf16, tag="sc")
                nc.scalar.activation(
                    sc[:], ps[:, 0:SLOTS],
                    mybir.ActivationFunctionType.Copy)
                if c + EQ_LEAD + 1 < NCH:
                    _emit_eq(c + EQ_LEAD + 1)
                proj_q.append((c, sc))
                if len(proj_q) > PROJ_DELAY:
                    _emit_proj(*proj_q.pop(0))
            while proj_q:
                _emit_proj(*proj_q.pop(0))
            y_sb = cpool.tile([P, NCH * NYC], f32)
            nc.vector.tensor_copy(y_sb[:], yp_all[:, 0:NCH * NYC])
            nc.sync.dma_start(out_r[:, :], y_sb[:])
    nc.finalize()
    return nc


def _prep(inputs):
    x = np.asarray(inputs["scalar_representation"], dtype=np.float32)
    idx = np.asarray(inputs["idx_m"]).astype(np.int64)
    W = np.asarray(inputs["W"], dtype=np.float32)
    b = np.asarray(inputs["b"], dtype=np.float32)

    # fold the bias into x: (x + v) . w0 == x . w0 + b0
    v = W[0] * (b[0] / np.dot(W[0], W[0]))

    IOTA_OFF = 0
    LIDX_OFF = SLOTS
    W0_OFF = LIDX_OFF + NCH * NB
    W0_OFF += W0_OFF % 2
    CW = W0_OFF + 4
    iota_row = np.tile(np.arange(BF, dtype=np.float32), NB).astype(
        ml_dtypes.float8_e3m4)
    groups = _chunk_groups(NCH)

    in_maps = []
    unpack = []
    for i in range(NCORES):
        idxc = idx[i * PC:(i + 1) * PC]
        q8 = np.zeros((NPAD, N_IN), dtype=ml_dtypes.float8_e3m4)
        q8[:PC] = (x[i * PC:(i + 1) * PC] + v).astype(ml_dtypes.float8_e3m4)

        kb = np.arange(NCH * NB, dtype=np.int64) * P
        valid = kb < PC
        base = np.zeros(NCH * NB, dtype=np.int64)
        base[valid] = idxc[kb[valid]]
        ke = np.minimum(kb + P - 1, PC - 1)
        span = np.zeros(NCH * NB, dtype=np.int64)
        span[valid] = idxc[ke[valid]] - base[valid] + 1
        assert span.max() <= BF, f"block span {span.max()} > BF={BF}"

        lidx = np.full(NPAD, -1.0, dtype=np.float32)
        lidx[:PC] = (idxc - np.repeat(base, P)[:PC]).astype(np.float32)
        lidx_t = lidx.reshape(NCH, NB, P).transpose(2, 0, 1).reshape(
            P, NCH * NB).astype(ml_dtypes.float8_e3m4)

        parts = []
        for gstart, gc in groups:
            blk = q8[gstart * A_max:(gstart + gc) * A_max]
            parts.append(np.ascontiguousarray(
                blk.reshape(gc, NB, P, N_IN).transpose(2, 0, 1, 3)
                   .reshape(gc * A_max, N_IN)))
        xw_i = np.concatenate(parts, axis=0)

        cst = np.zeros((P, CW), dtype=ml_dtypes.float8_e3m4)
        cst[:, IOTA_OFF:IOTA_OFF + SLOTS] = iota_row[None, :]
        cst[:, LIDX_OFF:LIDX_OFF + NCH * NB] = lidx_t
        w0b = W[0].astype(ml_dtypes.bfloat16)[:, None].view(
            ml_dtypes.float8_e3m4)
        cst[:, W0_OFF:W0_OFF + 2] = w0b
        in_maps.append({"xw": xw_i, "cst": np.ascontiguousarray(cst)})

        # unpack tables: (partition, column, molecule) per live slot
        nblk = int(valid.sum())
        sp = span[:nblk]
        tot = int(sp.sum())
        starts = np.zeros(nblk, dtype=np.int64)
        starts[1:] = np.cumsum(sp)[:-1]
        blk_of = np.repeat(np.arange(nblk, dtype=np.int64), sp)
        s_off = np.arange(tot, dtype=np.int64) - np.repeat(starts, sp)
        mol = np.repeat(base[:nblk], sp) + s_off
        slotfull = (blk_of % NB) * BF + s_off
        chunkc = blk_of // NB
        p_idx = slotfull % P
        col_idx = chunkc * NYC + slotfull // P
        unpack.append((p_idx, col_idx, mol))
    return in_maps, unpack


def _run(inputs, trace=False):
    from concourse import bass_utils

    in_maps, unpack = _prep(inputs)
    key = (NCH, BF)
    if key not in _graph_cache:
        _graph_cache[key] = _build_graph()
    nc = _graph_cache[key]

    res = bass_utils.run_bass_kernel_spmd(
        nc, in_maps, core_ids=list(range(NCORES)), trace=trace
    )
    y = np.zeros(N_MOL, dtype=np.float32)
    for i in range(NCORES):
        arr = res.results[i]["out"].reshape(P, NCH * NYC)
        p_idx, col_idx, mol = unpack[i]
        np.add.at(y, mol, arr[p_idx, col_idx])
    return y, res


def kernel(**inputs) -> np.ndarray:
    y, _ = _run(inputs, trace=False)
    return y
